# revision 1
# baseline (speedup 1.0000x reference)
"""Trainium2 kernel: per-pixel channel-mixing attention via temperature
interpolation (sigma-interp).

Math per pixel: out_i = sum_j sigma_i(k_j) q_j where sigma(t) = softmax(t*v)
over channels. sigma(t*v) is interpolated in the temperature t at M=12
per-pixel-scaled Chebyshev nodes t_m = Tk*u_m (Tk = max|k| per pixel):

    out_i = sum_m exp(u_m * Tk*v_i) * S_m,   S_m = R_m / G_m
    G_m   = sum_i exp(u_m * Tk*v_i)
    R_m   = sum_r Lc[r,m] * That_r,  That_r = sum_j T_r(k_j/Tk) q_j

Host sorts pixels by A = max|v|*max|k| (interp bandwidth); the hardest 128
per core go through an exact pair-grid tile instead.

Device layout (per core, 3968 interp pixels): channel-major g-split
[(2g,64ch) partitions, 1984 pixels free]. ACT: M exp passes (scale=u_m
immediate). PE: all channel reductions (zero-padded ones-block
stationaries accumulating into one PSUM tile), the Lc coefficient matmul,
per-node S broadcasts (delta stationaries), and the final node-sum
(identity accumulate). DVE: Chebyshev chain in fp16 @2x and per-node
eval multiplies (PSUM operand, 1x).
"""

import sys

sys.path.insert(0, "/opt/trn_rl_repo")

from contextlib import ExitStack

import ml_dtypes
import numpy as np

import concourse.bacc as bacc
import concourse.bass as bass
import concourse.tile as tile
from concourse import mybir
from concourse.bass_utils import run_bass_kernel_spmd

B, C, H, W = 2, 64, 128, 128
N_CORES = 8
NPIX = B * H * W            # 32768
M = 12                      # interp nodes / chebyshev terms
NEX_CORE = 128              # exact pixels per core
NEZ_CORE = NPIX // N_CORES - NEX_CORE   # 3968 interp pixels per core
FD = NEZ_CORE // 2          # 1984 pixels per g-half
R2 = 2 * M                  # psum rows: (2m+g)

FP32 = mybir.dt.float32
FP16 = mybir.dt.float16
BF16 = mybir.dt.bfloat16
EXP = mybir.ActivationFunctionType.Exp

U_NODES = np.cos(np.pi * np.arange(M) / (M - 1))   # cheb pts incl endpoints


def _lc_matrix():
    u = U_NODES
    Tn = np.cos(np.arange(M)[:, None] * np.arccos(u)[None, :])  # T_r(u_m)
    return np.linalg.inv(Tn.T)   # Lc[r, m]: L_m(t) = sum_r Lc[r,m] T_r(t)


def build_kernel():
    nc = bacc.Bacc(
        "TRN2",
        target_bir_lowering=False,
        debug=False,
        enable_asserts=False,
        num_devices=N_CORES,
    )
    vp = nc.dram_tensor("vp", [128, FD], FP32, kind="ExternalInput").ap()
    k2t = nc.dram_tensor("k2t", [128, FD], FP16, kind="ExternalInput").ap()
    qt = nc.dram_tensor("qt", [128, FD], FP16, kind="ExternalInput").ap()
    # stationaries / constants
    statRb = nc.dram_tensor("statRb", [128, M, R2], BF16, kind="ExternalInput").ap()
    statRh = nc.dram_tensor("statRh", [128, M, R2], FP16, kind="ExternalInput").ap()
    lcb = nc.dram_tensor("lcb", [R2, R2], FP16, kind="ExternalInput").ap()
    statB = nc.dram_tensor("statB", [R2, M, 128], BF16, kind="ExternalInput").ap()
    identb = nc.dram_tensor("identb", [128, 128], BF16, kind="ExternalInput").ap()
    # exact-tile inputs (pixel-major from host)
    vE = nc.dram_tensor("vE", [128, C], FP32, kind="ExternalInput").ap()
    kE = nc.dram_tensor("kE", [128, C], FP32, kind="ExternalInput").ap()
    qE = nc.dram_tensor("qE", [128, C], FP32, kind="ExternalInput").ap()

    outm = nc.dram_tensor("outm", [128, FD], FP16, kind="ExternalOutput").ap()
    oute = nc.dram_tensor("oute", [128, C], FP32, kind="ExternalOutput").ap()

    with tile.TileContext(nc) as tc, ExitStack() as ctx:
        sb = ctx.enter_context(tc.tile_pool(name="sb", bufs=1))
        sbw = ctx.enter_context(tc.tile_pool(name="sbw", bufs=1))
        sbp = ctx.enter_context(tc.tile_pool(name="sbp", bufs=2))
        red = ctx.enter_context(tc.tile_pool(name="red", bufs=1, space="PSUM"))
        evp = ctx.enter_context(tc.tile_pool(name="evp", bufs=2, space="PSUM"))
        acp = ctx.enter_context(tc.tile_pool(name="acp", bufs=1, space="PSUM"))

        v_t = sb.tile([128, FD], FP32)
        k2_t = sb.tile([128, FD], FP16)
        q_t = sb.tile([128, FD], FP16)
        sRb = sb.tile([128, M, R2], BF16)
        sRh = sb.tile([128, M, R2], FP16)
        lc_t = sb.tile([R2, R2], FP16)
        sB_t = sb.tile([R2, M, 128], BF16)
        id_t = sb.tile([128, 128], BF16)
        nc.sync.dma_start(out=v_t, in_=vp)
        nc.sync.dma_start(out=k2_t, in_=k2t)
        nc.sync.dma_start(out=q_t, in_=qt)
        nc.sync.dma_start(out=sRb, in_=statRb)
        nc.sync.dma_start(out=sRh, in_=statRh)
        nc.sync.dma_start(out=lc_t, in_=lcb)
        nc.sync.dma_start(out=sB_t, in_=statB)
        nc.sync.dma_start(out=id_t, in_=identb)

        # Reduce psum, column-halved: G rows [0:24], That rows [32:56].
        HFD = FD // 2
        LN = mybir.ActivationFunctionType.Ln

        # ---- chebyshev chain (fp16, full-width); T-matmuls per col-half ----
        Ws = []
        w_cur = sbw.tile([128, FD], FP16, tag="w1")
        nc.vector.scalar_tensor_tensor(
            out=w_cur, in0=k2_t, scalar=0.5, in1=q_t,
            op0=mybir.AluOpType.mult, op1=mybir.AluOpType.mult,
        )
        Ws.append(q_t)
        Ws.append(w_cur)
        w_prev = q_t
        for r in range(2, M):
            u_t = sbp.tile([128, FD], FP16, tag="u")
            nc.vector.tensor_mul(u_t, k2_t, w_cur)
            w_nxt = sbw.tile([128, FD], FP16, tag=f"w{r}")
            nc.vector.tensor_sub(w_nxt, u_t, w_prev)
            Ws.append(w_nxt)
            w_prev, w_cur = w_cur, w_nxt

        # ---- X grids (full-width ACT) ----
        Xs = []
        for m in range(M):
            xm = sbw.tile([128, FD], BF16, tag=f"x{m}")
            nc.scalar.activation(out=xm, in_=v_t, func=EXP, scale=float(U_NODES[m]))
            Xs.append(xm)

        # ---- per column-half: G/That reduce, R, S ----
        s_halves = []
        for h in range(2):
            hs = slice(h * HFD, (h + 1) * HFD)
            CHH0 = [0, 512, HFD]
            red_t = red.tile([56, HFD], FP32, tag="red")
            g_ps = red_t[0:R2, :]
            t_ps = red_t[32:32 + R2, :]
            for m in range(M):
                for a, b in zip(CHH0[:-1], CHH0[1:]):
                    nc.tensor.matmul(g_ps[:, a:b], sRb[:, m, :], Xs[m][:, hs][:, a:b],
                                     start=(m == 0), stop=(m == M - 1))
            for r in range(M):
                for a, b in zip(CHH0[:-1], CHH0[1:]):
                    nc.tensor.matmul(t_ps[:, a:b], sRh[:, r, :], Ws[r][:, hs][:, a:b],
                                     start=(r == 0), stop=(r == M - 1))
            lng = sbw.tile([R2, HFD], FP32, tag=f"lng{h}")
            nc.scalar.activation(out=lng, in_=g_ps, func=LN)
            ginv = sbw.tile([R2, HFD], FP32, tag=f"ginv{h}")
            nc.scalar.activation(out=ginv, in_=lng, func=EXP, scale=-1.0)
            t_sb = sbw.tile([R2, HFD], FP16, tag=f"tsb{h}")
            nc.scalar.copy(t_sb, t_ps)
            for a, b in zip(CHH0[:-1], CHH0[1:]):
                nc.tensor.matmul(g_ps[:, a:b], lc_t, t_sb[:, a:b], start=True, stop=True)
            s_th = sbw.tile([R2, HFD], BF16, tag=f"s{h}")
            nc.vector.tensor_mul(s_th, ginv, g_ps)
            s_halves.append(s_th)

        # ---- exact tile (pixel-major, pair-grid) ----
        v2 = sb.tile([128, C, 2], FP16)
        nc.scalar.copy(v2, vE_bcast(nc, sb, vE))
        kE16 = sb.tile([128, C], FP16)
        qE_t = sb.tile([128, C], FP32)
        nc.sync.dma_start(out=qE_t, in_=qE)
        nc.scalar.copy(kE16, kE_load(nc, sb, kE))
        P_t = sb.tile([128, C, C], FP16)
        k_op = bass.AP(
            tensor=kE16.tensor, offset=kE16.offset,
            ap=[kE16.ap[0], [0, C], [2, C // 2], [1, 2]],
        )
        v_op = bass.AP(
            tensor=v2.tensor, offset=v2.offset,
            ap=[v2.ap[0], [2, C], [0, C // 2], [1, 2]],
        )
        nc.vector.tensor_mul(P_t.rearrange("p i (jh jp) -> p i jh jp", jp=2), k_op, v_op)
        E_t = sb.tile([128, C, C], BF16)
        nc.scalar.activation(out=E_t, in_=P_t, func=EXP)
        G1 = sb.tile([128, C // 4, C], BF16)
        G2 = sb.tile([128, C // 4, C], BF16)
        nc.vector.tensor_add(G1, E_t[:, : C // 4, :], E_t[:, C // 4: C // 2, :])
        nc.vector.tensor_add(G2, E_t[:, C // 2: 3 * C // 4, :], E_t[:, 3 * C // 4:, :])
        nc.gpsimd.dma_start(out=G1, in_=G2, accum_op=mybir.AluOpType.add)
        nc.vector.tensor_add(G1[:, : C // 8, :], G1[:, : C // 8, :], G1[:, C // 8: C // 4, :])
        nc.vector.tensor_add(G1[:, : C // 16, :], G1[:, : C // 16, :], G1[:, C // 16: C // 8, :])
        d_t = sb.tile([128, C], FP32)
        nc.vector.tensor_reduce(
            out=d_t, in_=G1[:, : C // 16, :].transpose([0, 2, 1]),
            axis=mybir.AxisListType.X, op=mybir.AluOpType.add,
        )
        r_t = sb.tile([128, C], FP32)
        nc.vector.reciprocal(r_t, d_t)
        w16 = sb.tile([128, C], BF16)
        nc.vector.tensor_mul(w16, qE_t, r_t)
        Q4 = C // 4
        F1 = sb.tile([128, C, Q4], BF16)
        F2 = sb.tile([128, C, Q4], BF16)
        F3 = sb.tile([128, C, Q4], BF16)
        F4 = sb.tile([128, C, Q4], BF16)
        for fi, Fq in enumerate((F1, F2, F3, F4)):
            nc.vector.tensor_mul(
                Fq, E_t[:, :, fi * Q4: (fi + 1) * Q4],
                w16[:, None, fi * Q4: (fi + 1) * Q4].broadcast_to([128, C, Q4]),
            )
        nc.gpsimd.dma_start(out=F1, in_=F2, accum_op=mybir.AluOpType.add)
        nc.gpsimd.dma_start(out=F3, in_=F4, accum_op=mybir.AluOpType.add)
        nc.vector.tensor_add(F1, F1, F3)
        nc.vector.tensor_add(F1[:, :, : Q4 // 2], F1[:, :, : Q4 // 2], F1[:, :, Q4 // 2:])
        nc.vector.tensor_add(F1[:, :, : Q4 // 4], F1[:, :, : Q4 // 4], F1[:, :, Q4 // 4: Q4 // 2])
        oE = sb.tile([128, C], FP32)
        nc.vector.tensor_reduce(
            out=oE, in_=F1[:, :, : Q4 // 4],
            axis=mybir.AxisListType.X, op=mybir.AluOpType.add,
        )
        nc.sync.dma_start(out=oute, in_=oE)

        # ---- eval: out = sum_m X_m * bcast(S_m), accumulated on PE ----
        CHH = [0, 512, HFD]
        for half in range(2):
            sl = slice(half * HFD, (half + 1) * HFD)
            acc = acp.tile([128, HFD], FP32, tag="acc")
            for m in range(M):
                s_b = evp.tile([128, HFD], FP32, tag="sbps")
                for a, b2 in zip(CHH[:-1], CHH[1:]):
                    nc.tensor.matmul(s_b[:, a:b2], sB_t[:, m, :],
                                     s_halves[half][:, a:b2], start=True, stop=True)
                prod = sbp.tile([128, HFD], BF16, tag="prod")
                nc.vector.tensor_mul(prod, Xs[m][:, sl], s_b)
                for a, b2 in zip(CHH[:-1], CHH[1:]):
                    nc.tensor.matmul(acc[:, a:b2], id_t, prod[:, a:b2],
                                     start=(m == 0), stop=(m == M - 1))
            o_sb = sbp.tile([128, HFD], FP16, tag="osb")
            nc.scalar.copy(o_sb, acc)
            nc.sync.dma_start(out=outm[:, sl], in_=o_sb)

    nc.compile()
    return nc


def vE_bcast(nc, sb, vE):
    vE_t = sb.tile([128, C], FP32)
    nc.sync.dma_start(out=vE_t, in_=vE)
    return vE_t[:, :, None].broadcast_to([128, C, 2])


def kE_load(nc, sb, kE):
    kE_t = sb.tile([128, C], FP32)
    nc.sync.dma_start(out=kE_t, in_=kE)
    return kE_t


_NC_CACHE = None


def _get_nc():
    global _NC_CACHE
    if _NC_CACHE is None:
        _NC_CACHE = build_kernel()
    return _NC_CACHE


def _prep(x, y, z):
    """Host prep: sort by difficulty, shard, scale. Returns in_maps + meta."""
    q = np.ascontiguousarray(np.transpose(np.asarray(x), (0, 2, 3, 1))).reshape(-1, C)
    k = np.ascontiguousarray(np.transpose(np.asarray(y), (0, 2, 3, 1))).reshape(-1, C)
    v = np.ascontiguousarray(np.transpose(np.asarray(z), (0, 2, 3, 1))).reshape(-1, C)
    Tk = np.abs(k).max(axis=1)
    A = Tk * np.abs(v).max(axis=1)
    order = np.argsort(A, kind="stable")
    easy = order[: NEZ_CORE * N_CORES]
    hard = order[NEZ_CORE * N_CORES:]

    Lc = _lc_matrix()
    statRb = np.zeros((128, M, R2), np.float32)
    for m in range(M):
        for g in range(2):
            statRb[g * 64:(g + 1) * 64, m, 2 * m + g] = 1
    statRh = statRb.copy()
    lcb = np.zeros((R2, R2), np.float32)
    for r in range(M):
        for m in range(M):
            for g in range(2):
                lcb[2 * r + g, 2 * m + g] = Lc[r, m]
    statB = np.zeros((R2, M, 128), np.float32)
    for m in range(M):
        for g in range(2):
            statB[2 * m + g, m, g * 64:(g + 1) * 64] = 1
    identb = np.eye(128, dtype=np.float32)

    in_maps = []
    meta = []
    for c in range(N_CORES):
        ez = easy[c::N_CORES]
        hd = hard[c::N_CORES]
        vp_c = (Tk[ez, None] * v[ez]).astype(np.float32)     # [3968, 64]
        k2_c = (2.0 * k[ez] / Tk[ez, None]).astype(np.float16)
        q_c = q[ez].astype(np.float16)

        def cmaj(a2d, dt):
            # [3968, ch] -> [(2g,64ch), 1984] channel-major halves
            h0 = a2d[:FD].T
            h1 = a2d[FD:].T
            return np.ascontiguousarray(np.concatenate([h0, h1], axis=0)).astype(dt)

        in_maps.append({
            "vp": cmaj(vp_c, np.float32),
            "k2t": cmaj(k2_c, np.float16),
            "qt": cmaj(q_c, np.float16),
            "statRb": statRb.astype(ml_dtypes.bfloat16),
            "statRh": statRh.astype(np.float16),
            "lcb": lcb.astype(np.float16),
            "statB": statB.astype(ml_dtypes.bfloat16),
            "identb": identb.astype(ml_dtypes.bfloat16),
            "vE": v[hd].astype(np.float32),
            "kE": k[hd].astype(np.float32),
            "qE": q[hd].astype(np.float32),
        })
        meta.append((ez, hd))
    return in_maps, meta


def kernel(x, y, z):
    nc = _get_nc()
    in_maps, meta = _prep(x, y, z)
    res = run_bass_kernel_spmd(nc, in_maps, core_ids=list(range(N_CORES)))
    out = np.empty((NPIX, C), np.float32)
    for c in range(N_CORES):
        ez, hd = meta[c]
        om = res.results[c]["outm"].astype(np.float32)   # [(2g,64), FD]
        out[ez[:FD]] = om[:64].T
        out[ez[FD:]] = om[64:].T
        out[hd] = res.results[c]["oute"]
    return np.ascontiguousarray(
        np.transpose(out.reshape(B, H, W, C), (0, 3, 1, 2))
    ).astype(np.float32)



# revision 2
# speedup vs baseline: 1.1470x; 1.1470x over previous
"""Trainium2 kernel: per-pixel channel-mixing attention via temperature
interpolation (sigma-interp), v2.

Math per pixel: out_i = sum_j sigma_i(k_j) q_j where sigma(t) = softmax(t*v)
over channels. sigma(t*v) is interpolated in the temperature t at M=11
per-pixel-scaled Chebyshev nodes t_m = Tk*u_m (Tk = max|k| per pixel):

    out_i = sum_m exp(u_m * Tk*v_i) * S_m,   S_m = R_m / G_m
    G_m   = sum_i exp(u_m * Tk*v_i)
    R_m   = sum_r Lc[r,m] * That_r,  That_r = sum_j T_r(k_j/Tk) q_j

M=11 includes the center node u=0 whose grid is identically 1: its exp
pass and eval multiply are skipped (G via a ones-tile reduce; its eval
contribution is a single broadcast-accumulate matmul into the output).

Host sorts pixels by A = max|v|*max|k| (interp bandwidth); the hardest 128
per core go through an exact pair-grid tile instead.

v2 perf structure vs v1:
  - PE warmup matmuls at t=0 flip the HAM clock gate to 2.4 GHz before the
    real matmul stream starts (v1 ran most matmuls at the cold 1.2 GHz).
  - no Ln activations: 1/G via the custom-DVE reciprocal_approx_fast, so
    the ACT table is loaded once (v1 thrashed exp<->ln tables 5x) and the
    S critical path between analysis and eval no longer stalls the PE.
  - U1 = khat*q comes from the host (kills a full-width DVE pass).
  - reduce PSUM double-buffered per column-half so both halves' analysis
    accumulate concurrently.
  - eval: per-node S broadcast (PE) -> fp16 SBUF copy (ACT) -> 2x DVE
    multiply -> PE identity-accumulate; X grids in fp16.
"""

import sys

sys.path.insert(0, "/opt/trn_rl_repo")

from contextlib import ExitStack

import ml_dtypes
import numpy as np

import concourse.bacc as bacc
import concourse.bass as bass
import concourse.tile as tile
from concourse import mybir
from concourse.bass_utils import run_bass_kernel_spmd

B, C, H, W = 2, 64, 128, 128
N_CORES = 8
NPIX = B * H * W            # 32768
M = 11                      # interp nodes / chebyshev terms (odd: u=0 free)
MC = M // 2                 # index of the center node (u=0)
NEX_CORE = 128              # exact pixels per core
NEZ_CORE = NPIX // N_CORES - NEX_CORE   # 3968 interp pixels per core
FD = NEZ_CORE // 2          # 1984 pixels per g-half
HFD = FD // 2               # 992 pixels per column-half
R2 = 2 * M                  # psum rows: (2m+g)
N_WARM = 26                 # PE warmup matmuls

FP32 = mybir.dt.float32
FP16 = mybir.dt.float16
BF16 = mybir.dt.bfloat16
EXP = mybir.ActivationFunctionType.Exp

U_NODES = np.cos(np.pi * np.arange(M) / (M - 1))   # cheb pts incl endpoints


def _lc_matrix():
    u = U_NODES
    Tn = np.cos(np.arange(M)[:, None] * np.arccos(np.clip(u, -1, 1))[None, :])
    return np.linalg.inv(Tn.T)   # Lc[r, m]: L_m(t) = sum_r Lc[r,m] T_r(t)


def build_kernel():
    nc = bacc.Bacc(
        "TRN2",
        target_bir_lowering=False,
        debug=False,
        enable_asserts=False,
        num_devices=N_CORES,
    )
    vp = nc.dram_tensor("vp", [128, FD], FP32, kind="ExternalInput").ap()
    k2t = nc.dram_tensor("k2t", [128, FD], FP16, kind="ExternalInput").ap()
    qt = nc.dram_tensor("qt", [128, FD], FP16, kind="ExternalInput").ap()
    u1t = nc.dram_tensor("u1t", [128, FD], FP16, kind="ExternalInput").ap()
    # stationaries / constants
    statR = nc.dram_tensor("statR", [128, M, R2], FP16, kind="ExternalInput").ap()
    lcb = nc.dram_tensor("lcb", [R2, R2], FP16, kind="ExternalInput").ap()
    statB = nc.dram_tensor("statB", [R2, M, 128], FP16, kind="ExternalInput").ap()
    identb = nc.dram_tensor("identb", [128, 128], FP16, kind="ExternalInput").ap()
    # exact-tile inputs (pixel-major from host)
    vE = nc.dram_tensor("vE", [128, C], FP32, kind="ExternalInput").ap()
    kE = nc.dram_tensor("kE", [128, C], FP32, kind="ExternalInput").ap()
    qE = nc.dram_tensor("qE", [128, C], FP32, kind="ExternalInput").ap()

    outm = nc.dram_tensor("outm", [128, FD], FP16, kind="ExternalOutput").ap()
    oute = nc.dram_tensor("oute", [128, C], FP32, kind="ExternalOutput").ap()

    CH = [0, 512, HFD]          # psum-bank chunks within a column-half

    with tile.TileContext(nc) as tc, ExitStack() as ctx:
        sb = ctx.enter_context(tc.tile_pool(name="sb", bufs=1))
        sbw = ctx.enter_context(tc.tile_pool(name="sbw", bufs=1))
        sbp = ctx.enter_context(tc.tile_pool(name="sbp", bufs=2))
        red = ctx.enter_context(tc.tile_pool(name="red", bufs=2, space="PSUM"))
        evp = ctx.enter_context(tc.tile_pool(name="evp", bufs=1, space="PSUM"))
        acp = ctx.enter_context(tc.tile_pool(name="acp", bufs=1, space="PSUM"))

        # ---- PE warmup: junk matmuls from t=0 flip HAM to 2.4 GHz ----
        wu = sbw.tile([128, 512], FP16, tag="wu")
        nc.gpsimd.memset(wu, 1.0)
        wu_ps = acp.tile([128, HFD], FP32, tag="acc")
        for i in range(N_WARM):
            nc.tensor.matmul(wu_ps[:, 0:496], wu[:, 0:128], wu[:, 0:496],
                             start=True, stop=True)

        # ---- input DMAs ----
        v_t = sb.tile([128, FD], FP32)
        k2_t = sb.tile([128, FD], FP16)
        q_t = sb.tile([128, FD], FP16)
        u1_t = sb.tile([128, FD], FP16)
        sR = sb.tile([128, M, R2], FP16)
        lc_t = sb.tile([R2, R2], FP16)
        sB_t = sb.tile([R2, M, 128], FP16)
        id_t = sb.tile([128, 128], FP16)
        nc.sync.dma_start(out=v_t[:, :FD // 2], in_=vp[:, :FD // 2])
        nc.sync.dma_start(out=v_t[:, FD // 2:], in_=vp[:, FD // 2:])
        nc.sync.dma_start(out=k2_t, in_=k2t)
        nc.sync.dma_start(out=u1_t, in_=u1t)
        nc.sync.dma_start(out=q_t, in_=qt)
        nc.gpsimd.dma_start(out=sR, in_=statR)
        nc.gpsimd.dma_start(out=lc_t, in_=lcb)
        nc.gpsimd.dma_start(out=sB_t, in_=statB)
        nc.gpsimd.dma_start(out=id_t, in_=identb)

        # ones grid stands in for X at the center node (exp(0) = 1)
        ones_t = sbw.tile([128, FD], FP16, tag="ones")
        nc.vector.memset(ones_t, 1.0)

        # dummy activation: pull the exp table load off the critical path
        dum = sbw.tile([1, 16], FP32, tag="dum")
        nc.scalar.activation(out=dum, in_=wu[0:1, 0:16], func=EXP)

        # ---- X grids (ACT, fp16) and chebyshev chain (DVE, fp16) ----
        Xs = {}
        for m in range(M):
            if m == MC:
                Xs[m] = ones_t
                continue
            xm = sbw.tile([128, FD], FP16, tag=f"x{m}")
            nc.scalar.activation(out=xm, in_=v_t, func=EXP, scale=float(U_NODES[m]))
            Xs[m] = xm

        Ws = [q_t, u1_t]
        w_prev, w_cur = q_t, u1_t
        for r in range(2, M):
            u_t = sbp.tile([128, FD], FP16, tag="u")
            nc.vector.tensor_mul(u_t, k2_t, w_cur)
            w_nxt = sbw.tile([128, FD], FP16, tag=f"w{r}")
            nc.vector.tensor_sub(w_nxt, u_t, w_prev)
            Ws.append(w_nxt)
            w_prev, w_cur = w_cur, w_nxt

        # ---- analysis reductions: G rows [0:R2], That rows [32:32+R2] ----
        red_ts = [red.tile([32 + R2, HFD], FP32, tag="red", name=f"red{h}")
                  for h in range(2)]
        for m in range(M):
            for h in range(2):
                hs = slice(h * HFD, (h + 1) * HFD)
                g_ps = red_ts[h][0:R2, :]
                t_ps = red_ts[h][32:32 + R2, :]
                for a, b in zip(CH[:-1], CH[1:]):
                    nc.tensor.matmul(g_ps[:, a:b], sR[:, m, :], Xs[m][:, hs][:, a:b],
                                     start=(m == 0), stop=(m == M - 1))
                for a, b in zip(CH[:-1], CH[1:]):
                    nc.tensor.matmul(t_ps[:, a:b], sR[:, m, :], Ws[m][:, hs][:, a:b],
                                     start=(m == 0), stop=(m == M - 1))

        # ---- S = (Lc That) / G per half ----
        s_halves = []
        for h in range(2):
            g_ps = red_ts[h][0:R2, :]
            t_ps = red_ts[h][32:32 + R2, :]
            ginv = sbw.tile([R2, HFD], FP32, tag=f"ginv{h}")
            nc.vector.reciprocal_approx_fast(out=ginv, in_=g_ps)
            t_sb = sbw.tile([R2, HFD], FP16, tag=f"tsb{h}")
            nc.scalar.copy(t_sb, t_ps)
            for a, b in zip(CH[:-1], CH[1:]):
                nc.tensor.matmul(g_ps[:, a:b], lc_t, t_sb[:, a:b],
                                 start=True, stop=True)
            s_th = sbw.tile([R2, HFD], FP16, tag=f"s{h}")
            nc.vector.tensor_mul(s_th, ginv, g_ps)
            s_halves.append(s_th)

        # ---- exact tile (pixel-major, pair-grid) ----
        v2 = sb.tile([128, C, 2], FP16)
        nc.scalar.copy(v2, vE_bcast(nc, sb, vE))
        kE16 = sb.tile([128, C], FP16)
        qE_t = sb.tile([128, C], FP32)
        nc.sync.dma_start(out=qE_t, in_=qE)
        nc.scalar.copy(kE16, kE_load(nc, sb, kE))
        P_t = sb.tile([128, C, C], FP16)
        k_op = bass.AP(
            tensor=kE16.tensor, offset=kE16.offset,
            ap=[kE16.ap[0], [0, C], [2, C // 2], [1, 2]],
        )
        v_op = bass.AP(
            tensor=v2.tensor, offset=v2.offset,
            ap=[v2.ap[0], [2, C], [0, C // 2], [1, 2]],
        )
        nc.vector.tensor_mul(P_t.rearrange("p i (jh jp) -> p i jh jp", jp=2), k_op, v_op)
        E_t = sb.tile([128, C, C], BF16)
        nc.scalar.activation(out=E_t, in_=P_t, func=EXP)
        G1 = sb.tile([128, C // 4, C], BF16)
        G2 = sb.tile([128, C // 4, C], BF16)
        nc.vector.tensor_add(G1, E_t[:, : C // 4, :], E_t[:, C // 4: C // 2, :])
        nc.vector.tensor_add(G2, E_t[:, C // 2: 3 * C // 4, :], E_t[:, 3 * C // 4:, :])
        nc.gpsimd.dma_start(out=G1, in_=G2, accum_op=mybir.AluOpType.add)
        nc.vector.tensor_add(G1[:, : C // 8, :], G1[:, : C // 8, :], G1[:, C // 8: C // 4, :])
        nc.vector.tensor_add(G1[:, : C // 16, :], G1[:, : C // 16, :], G1[:, C // 16: C // 8, :])
        d_t = sb.tile([128, C], FP32)
        nc.vector.tensor_reduce(
            out=d_t, in_=G1[:, : C // 16, :].transpose([0, 2, 1]),
            axis=mybir.AxisListType.X, op=mybir.AluOpType.add,
        )
        r_t = sb.tile([128, C], FP32)
        nc.vector.reciprocal_approx_fast(out=r_t, in_=d_t)
        w16 = sb.tile([128, C], BF16)
        nc.vector.tensor_mul(w16, qE_t, r_t)
        Q4 = C // 4
        F1 = sb.tile([128, C, Q4], BF16)
        F2 = sb.tile([128, C, Q4], BF16)
        F3 = sb.tile([128, C, Q4], BF16)
        F4 = sb.tile([128, C, Q4], BF16)
        for fi, Fq in enumerate((F1, F2, F3, F4)):
            nc.vector.tensor_mul(
                Fq, E_t[:, :, fi * Q4: (fi + 1) * Q4],
                w16[:, None, fi * Q4: (fi + 1) * Q4].broadcast_to([128, C, Q4]),
            )
        nc.gpsimd.dma_start(out=F1, in_=F2, accum_op=mybir.AluOpType.add)
        nc.gpsimd.dma_start(out=F3, in_=F4, accum_op=mybir.AluOpType.add)
        nc.vector.tensor_add(F1, F1, F3)
        nc.vector.tensor_add(F1[:, :, : Q4 // 2], F1[:, :, : Q4 // 2], F1[:, :, Q4 // 2:])
        nc.vector.tensor_add(F1[:, :, : Q4 // 4], F1[:, :, : Q4 // 4], F1[:, :, Q4 // 4: Q4 // 2])
        oE = sb.tile([128, C], FP32)
        nc.vector.tensor_reduce(
            out=oE, in_=F1[:, :, : Q4 // 4],
            axis=mybir.AxisListType.X, op=mybir.AluOpType.add,
        )
        nc.sync.dma_start(out=oute, in_=oE)

        # ---- eval: out = sum_m X_m * bcast(S_m), accumulated on PE ----
        for half in range(2):
            sl = slice(half * HFD, (half + 1) * HFD)
            acc = acp.tile([128, HFD], FP32, tag="acc", name=f"acc{half}")
            for m in range(M):
                if m == MC:
                    # center node: X == 1, contribution is bcast(S_m) itself
                    for a, b in zip(CH[:-1], CH[1:]):
                        nc.tensor.matmul(acc[:, a:b], sB_t[:, m, :],
                                         s_halves[half][:, a:b],
                                         start=False, stop=False)
                    continue
                s_b = evp.tile([128, HFD], FP32, tag="sbps", name=f"sb{half}_{m}")
                for a, b in zip(CH[:-1], CH[1:]):
                    nc.tensor.matmul(s_b[:, a:b], sB_t[:, m, :],
                                     s_halves[half][:, a:b], start=True, stop=True)
                s_bs = sbp.tile([128, HFD], FP16, tag="sbs")
                nc.scalar.copy(s_bs, s_b)
                prod = sbp.tile([128, HFD], FP16, tag="prod")
                nc.vector.tensor_mul(prod, Xs[m][:, sl], s_bs)
                for a, b in zip(CH[:-1], CH[1:]):
                    nc.tensor.matmul(acc[:, a:b], id_t, prod[:, a:b],
                                     start=(m == 0), stop=(m == M - 1))
            o_sb = sbp.tile([128, HFD], FP16, tag="osb")
            nc.scalar.copy(o_sb, acc)
            nc.sync.dma_start(out=outm[:, sl], in_=o_sb)

    nc.compile()
    return nc


def vE_bcast(nc, sb, vE):
    vE_t = sb.tile([128, C], FP32)
    nc.sync.dma_start(out=vE_t, in_=vE)
    return vE_t[:, :, None].broadcast_to([128, C, 2])


def kE_load(nc, sb, kE):
    kE_t = sb.tile([128, C], FP32)
    nc.sync.dma_start(out=kE_t, in_=kE)
    return kE_t


_NC_CACHE = None


def _get_nc():
    global _NC_CACHE
    if _NC_CACHE is None:
        _NC_CACHE = build_kernel()
    return _NC_CACHE


def _prep(x, y, z):
    """Host prep: sort by difficulty, shard, scale. Returns in_maps + meta."""
    q = np.ascontiguousarray(np.transpose(np.asarray(x), (0, 2, 3, 1))).reshape(-1, C)
    k = np.ascontiguousarray(np.transpose(np.asarray(y), (0, 2, 3, 1))).reshape(-1, C)
    v = np.ascontiguousarray(np.transpose(np.asarray(z), (0, 2, 3, 1))).reshape(-1, C)
    Tk = np.abs(k).max(axis=1)
    A = Tk * np.abs(v).max(axis=1)
    order = np.argsort(A, kind="stable")
    easy = order[: NEZ_CORE * N_CORES]
    hard = order[NEZ_CORE * N_CORES:]

    Lc = _lc_matrix()
    statR = np.zeros((128, M, R2), np.float32)
    for m in range(M):
        for g in range(2):
            statR[g * 64:(g + 1) * 64, m, 2 * m + g] = 1
    lcb = np.zeros((R2, R2), np.float32)
    for r in range(M):
        for m in range(M):
            for g in range(2):
                lcb[2 * r + g, 2 * m + g] = Lc[r, m]
    statB = np.zeros((R2, M, 128), np.float32)
    for m in range(M):
        for g in range(2):
            statB[2 * m + g, m, g * 64:(g + 1) * 64] = 1
    identb = np.eye(128, dtype=np.float32)

    in_maps = []
    meta = []
    for c in range(N_CORES):
        ez = easy[c::N_CORES]
        hd = hard[c::N_CORES]
        kh = k[ez] / Tk[ez, None]
        vp_c = (Tk[ez, None] * v[ez]).astype(np.float32)     # [3968, 64]
        k2_c = (2.0 * kh).astype(np.float16)
        q_c = q[ez].astype(np.float16)
        u1_c = (kh * q[ez]).astype(np.float16)

        def cmaj(a2d, dt):
            # [3968, ch] -> [(2g,64ch), 1984] channel-major halves
            h0 = a2d[:FD].T
            h1 = a2d[FD:].T
            return np.ascontiguousarray(np.concatenate([h0, h1], axis=0)).astype(dt)

        in_maps.append({
            "vp": cmaj(vp_c, np.float32),
            "k2t": cmaj(k2_c, np.float16),
            "qt": cmaj(q_c, np.float16),
            "u1t": cmaj(u1_c, np.float16),
            "statR": statR.astype(np.float16),
            "lcb": lcb.astype(np.float16),
            "statB": statB.astype(np.float16),
            "identb": identb.astype(np.float16),
            "vE": v[hd].astype(np.float32),
            "kE": k[hd].astype(np.float32),
            "qE": q[hd].astype(np.float32),
        })
        meta.append((ez, hd))
    return in_maps, meta


def kernel(x, y, z):
    nc = _get_nc()
    in_maps, meta = _prep(x, y, z)
    res = run_bass_kernel_spmd(nc, in_maps, core_ids=list(range(N_CORES)))
    out = np.empty((NPIX, C), np.float32)
    for c in range(N_CORES):
        ez, hd = meta[c]
        om = res.results[c]["outm"].astype(np.float32)   # [(2g,64), FD]
        out[ez[:FD]] = om[:64].T
        out[ez[FD:]] = om[64:].T
        out[hd] = res.results[c]["oute"]
    return np.ascontiguousarray(
        np.transpose(out.reshape(B, H, W, C), (0, 3, 1, 2))
    ).astype(np.float32)
